# revision 25
# baseline (speedup 1.0000x reference)
"""Trainium2 Bass kernel for capsule dynamic routing (nn_Capsule) — v3.

Reference (per batch item b):
    u = x_b @ W; logits = 0
    for i in 4:
        c = softmax(logits, axis=capsule)
        t_j = sum_s c[s,j] * u[s, j*64:(j+1)*64]; v = squash(t)
        if i < 3: logits[s,j] += u[s, jblk] . v_j

Never materializes u (linearity):
    y_j   = sum_s c[s,j] x_s            y-GEMM   (c stationary, col-tiled)
    t     = W^T y^T                     t-GEMM   (w16 stationary per-slice)
    P^T   = Vblk^T W^T                  P-GEMM   (vblk stationary, block-diag)
    upd^T = P^T X                       upd-GEMM (P slices stationary, col-tiled)

v3 vs v2 (357us) / v1 (335us):
  - squash rsqrt on DVE (bitcast seed + 2 Newton steps, sign-folded into
    the iteration): ScalarE runs only Copy+Exp -> exactly ONE ACT table
    load for the whole kernel (v1/v2 thrashed sqrt|ln<->exp sets).
  - output v transposed on PE before the store so the final DMA writes
    512B-contiguous runs (v1/v2 scattered 4B writes burned ~25us of tail).
  - per-iteration stages split in halves (y/t by h-half, P/upd by h-half,
    u-evac/softmax per batch-group) and emitted interleaved so ScalarE
    evacs + sync-queue DMA transposes overlap PE instead of stalling it.
  - scalar queue issues NO DMA mid-iteration (ACTIVATE only); all
    transposes ride the sync queue; loads use 4 independent staging tiles
    (pool double-buffering raced on HW in v2 - distinct tags only).
  - all input casts f32->f16 on DVE (tensor_copy), not ScalarE.

HW lessons kept:
  - DVE copy PSUM(f32)->SBUF(f16) kills the device; PSUM->f16 casts go
    through ScalarE activation(Copy).
  - matmul start=True lazily zeroes the whole 2KB PSUM bank: accumulation
    groups must own a (partition-range x bank) region exclusively;
    partition-disjoint groups interleave with skip_group_check=True;
    column-disjoint writes into one bank are fine after the first
    start=True (has_written is per-element).
  - PSUM tiles that tiny matmuls write are padded to a full bank so pool
    neighbors never share a bank with an accumulating matmul.
  - nc.vector.memset on f16 tiles is unreliable: constants come from host.
"""
import numpy as np
from contextlib import ExitStack

import concourse.bass as bass
import concourse.bacc as bacc
import concourse.tile as tile
from concourse import mybir
from concourse.bass_utils import run_bass_kernel_spmd

f16 = mybir.dt.float16
f32 = mybir.dt.float32
i32 = mybir.dt.int32
COPY = mybir.ActivationFunctionType.Copy
EXP = mybir.ActivationFunctionType.Exp
MULT = mybir.AluOpType.mult
SUB = mybir.AluOpType.subtract
ADD = mybir.AluOpType.add
SHR = mybir.AluOpType.logical_shift_right

S, B, H = 512, 64, 1024
NCAP, DCAP = 16, 64
ROUTINGS = 4
N_CORES = 8
BL = B // N_CORES          # 8 batch items per core
SC = S // 128              # 4 s-chunks
HC = H // 128              # 8 h-chunks
OC = H // 128              # 8 o-chunks (o = NCAP*DCAP = 1024)
MAGIC = 0x5EF759DF         # rsqrt seed for h = s/2: 0x5f3759df - (1<<22)


def _act_copy(nc, out, in_):
    nc.scalar.activation(out=out, in_=in_, func=COPY, scale=1.0, alpha=0.0)


def _build_kernel(tc, out_d, x_d, xt_d, w_d, wt_d, c0_d, logits_d, vblk_d,
                  ones2_d, o2t_d, zeros_d, magic_d, dbg=None):
    nc = tc.nc
    ctx = ExitStack()
    const = ctx.enter_context(tc.tile_pool(name="const", bufs=1))
    work = ctx.enter_context(tc.tile_pool(name="work", bufs=1))
    small = ctx.enter_context(tc.tile_pool(name="small", bufs=2))
    ps_big = ctx.enter_context(tc.tile_pool(name="ps_big", bufs=2,
                                            space="PSUM"))
    ps_u = ctx.enter_context(tc.tile_pool(name="ps_u", bufs=2, space="PSUM"))
    ps_sm = ctx.enter_context(tc.tile_pool(name="ps_sm", bufs=2,
                                           space="PSUM"))

    # ---------- persistent tensors ----------
    x16 = const.tile([128, BL, SC, 1024], f16)      # (s_loc, b, sc, h)
    xt16 = const.tile([128, BL, HC, 512], f16)    # (h_loc, b, hc, s)
    w16 = const.tile([128, HC, 1024], f16)          # (h_loc, hc, o)
    wt16 = const.tile([128, OC, 1024], f16)         # (o_loc, oc, h)
    c16 = const.tile([128, SC, BL, 32], f16)        # coeffs, cols 16-31 zero
    logits = const.tile([128, SC, BL, NCAP], f32)
    vblk = const.tile([128, OC, 128], f16)          # block-diag v, bj dense
    ones2 = const.tile([128, 2], f16)               # [[1;0],[0;1]] halves
    o2t = const.tile([2, 128], f32)                 # broadcast helper
    zeros = const.tile([128, 128], f16)             # zero-weight psum fill
    magic = const.tile([2, 1], i32)                 # rsqrt seed constant

    # ---------- loads (all layouts host-prepped f16, raw DMAs) ----------
    # everything on the two HWDGE queues (gpsimd DMA completion raced
    # downstream consumers on HW).  zeros first: PE warm-up needs it.
    nc.sync.dma_start(out=zeros[:], in_=zeros_d[:])
    nc.sync.dma_start(out=ones2[:], in_=ones2_d[:])
    nc.sync.dma_start(out=o2t[:], in_=o2t_d[:])
    nc.sync.dma_start(out=magic[:], in_=magic_d[:])
    nc.scalar.dma_start(out=c16[:], in_=c0_d[:])
    nc.scalar.dma_start(out=logits[:], in_=logits_d[:])
    nc.scalar.dma_start(out=vblk[:], in_=vblk_d[:])
    # per-batch chunks so iteration-0 y-GEMM starts as batches land.
    # sync: x b0-3 then w16 then x b4-7? no -- x first on both queues so
    # y finishes early; w/wt right after (t/P of iter 0 need them later).
    for b in range(4):
        nc.sync.dma_start(out=x16[:, b], in_=x_d[:, b])
        nc.scalar.dma_start(out=x16[:, b + 4], in_=x_d[:, b + 4])
    nc.sync.dma_start(out=w16[:], in_=w_d[:])
    nc.scalar.dma_start(out=wt16[:], in_=wt_d[:])
    for b in range(4):
        nc.sync.dma_start(out=xt16[:, b], in_=xt_d[:, b])
        nc.scalar.dma_start(out=xt16[:, b + 4], in_=xt_d[:, b + 4])

    def _warm(ps_tile, n):
        # zero-weight matmuls into a psum tile whose next real producer
        # begins with start=True (which wipes the bank): pure HAM fuel
        # that runs during the dependency-wait gap before the stage.
        for k in range(n):
            nc.tensor.matmul(ps_tile[:, 0:512], zeros[:],
                             x16[:, 0, 0, 0:512],
                             start=(k == 0), stop=False,
                             skip_group_check=True)

    for it in range(ROUTINGS):
        last = it == ROUTINGS - 1

        # ---------- y = C^T X, h-halves pipelined ----------
        y_ps = [ps_big.tile([128, 1024], f32, tag="big", name=f"y{it}_{g}")
                for g in range(2)]
        _warm(y_ps[0], 24 if it == 0 else 10)
        y_sb = work.tile([128, 2, 1024], f16, tag="y_sb")
        yt = work.tile([128, HC, 256], f16, tag="yt")
        for half in range(2):
            hs = slice(512 * half, 512 * half + 512)
            for g in range(2):
                for b_ in range(4):
                    b = 4 * g + b_
                    for sc in range(SC):
                        nc.tensor.matmul(
                            y_ps[g][32 * b_:32 * b_ + 32, hs],
                            c16[:, sc, b, :],
                            x16[:, b, sc, hs],
                            start=(sc == 0), stop=(sc == SC - 1),
                            skip_group_check=True,
                            tile_position=(0, 32 * b_))
            for g in range(2):
                _act_copy(nc, y_sb[:, g, hs], y_ps[g][:, hs])
                nc.sync.dma_start_transpose(
                    yt[:, 4 * half:4 * half + 4, 128 * g:128 * g + 128],
                    y_sb[:, g, hs])
        # densify (drop j-pad): weights APs allow only one free dim
        yt_dn = work.tile([128, HC, 128], f16, tag="yt_dn")
        ytv = yt.rearrange("p hc (g b_ jp) -> p hc g b_ jp", g=2, jp=32)
        dnv = yt_dn.rearrange("p hc (g b_ j) -> p hc g b_ j", g=2, j=16)
        for g in range(2):
            nc.vector.tensor_copy(dnv[:, :, g], ytv[:, :, g, :, 0:16])

        # ---------- t^T = y W: out (bj dense, o), yt stationary ----------
        t_ps = ps_big.tile([128, 1024], f32, tag="big", name=f"t{it}")
        _warm(t_ps, 6)
        t16_sb = work.tile([128, 1024], f16, tag="t16_sb")
        tT = work.tile([128, OC, 128], f16, tag="tT")
        for half in range(2):
            hs = slice(512 * half, 512 * half + 512)
            for hcq in range(2):
                for hc in range(4 * hcq, 4 * hcq + 4):
                    nc.tensor.matmul(
                        t_ps[:, hs],
                        yt_dn[:, hc, :],
                        w16[:, hc, hs],
                        start=(hcq == 0 and hc == 0),
                        stop=(hcq == 1 and hc == 7),
                        skip_group_check=True)
            _act_copy(nc, t16_sb[:, hs], t_ps[:, hs])
            nc.sync.dma_start_transpose(tT[:, 4 * half:4 * half + 4, :],
                                        t16_sb[:, hs])

        # ---------- extract block-diag: t_sb (e*64+d, b*8+oc) ----------
        t_sb = small.tile([128, 64], f16, tag="t_sb")
        dv = t_sb.rearrange("p (b oc) -> p b oc", oc=OC)
        for oc in range(OC):
            tv = tT[:, oc, :].rearrange("p (b j) -> p b j", j=16)
            nc.vector.tensor_copy(dv[0:64, :, oc], tv[0:64, :, 2 * oc])
            nc.vector.tensor_copy(dv[64:128, :, oc],
                                  tv[64:128, :, 2 * oc + 1])

        # ---------- squash: rs = rsqrt(sum_d t^2 + eps) on DVE ----------
        t2 = small.tile([128, 64], f16, tag="t2")
        nc.vector.tensor_mul(t2[:], t_sb[:], t_sb[:])
        sq_ps = ps_sm.tile([2, 512], f32, tag="sm", name=f"sq{it}")
        nc.tensor.matmul(sq_ps[:, 0:64], ones2[:], t2[:])
        h_sb = small.tile([2, 64], f32, tag="h_sb")
        nc.vector.tensor_scalar(out=h_sb[:], in0=sq_ps[:, 0:64],
                                scalar1=0.5, scalar2=5e-8, op0=MULT, op1=ADD)
        ri = small.tile([2, 64], i32, tag="ri")
        nc.vector.tensor_scalar(out=ri[:], in0=h_sb.bitcast(i32),
                                scalar1=1, scalar2=None, op0=SHR)
        r0 = small.tile([2, 64], f32, tag="r0")
        nc.vector.tensor_tensor(out=r0.bitcast(i32),
                                in0=magic.broadcast_to([2, 64]),
                                in1=ri[:], op=SUB)
        # Newton x2 with sign fold: r' = (h*r*r - 1.5) * r  (negates once)
        rr = small.tile([2, 64], f32, tag="rr")
        rs = small.tile([2, 64], f32, tag="rs")
        nc.vector.tensor_mul(rr[:], r0[:], r0[:])
        nc.vector.tensor_mul(rr[:], rr[:], h_sb[:])
        nc.vector.scalar_tensor_tensor(out=rs[:], in0=rr[:], scalar=1.5,
                                       in1=r0[:], op0=SUB, op1=MULT)
        nc.vector.tensor_mul(rr[:], rs[:], rs[:])
        nc.vector.tensor_mul(rr[:], rr[:], h_sb[:])
        nc.vector.scalar_tensor_tensor(out=rs[:], in0=rr[:], scalar=1.5,
                                       in1=rs[:], op0=SUB, op1=MULT)
        # broadcast rs (2,64) -> (128,64): bc[p,n] = rs[p//64, n]
        bc_ps = ps_sm.tile([128, 512], f32, tag="sm", name=f"bc{it}")
        nc.tensor.matmul(bc_ps[:, 0:64], o2t[:], rs[:])

        if last:
            v32 = small.tile([128, 64], f32, tag="v32")
            nc.vector.tensor_mul(v32[:], t_sb[:], bc_ps[:, 0:64])
            # raw dump; host reorders (out[b,2oc+e,d] = v32[e*64+d, b*8+oc])
            nc.sync.dma_start(out=out_d, in_=v32[:])
            break

        # ---------- scatter v = t*bc into block-diag vblk (fused) -------
        # vblk[e*64+d, oc, b*16+2oc+e] = t_sb[e*64+d, b*8+oc] * bc[...]
        pitch = vblk[:].ap[0][0]
        for e in range(2):
            dst = bass.AP(tensor=vblk[:].tensor,
                          offset=64 * e * pitch + e,
                          ap=[[pitch, 64], [130, OC], [16, BL]])
            nc.vector.tensor_tensor(
                out=dst,
                in0=t_sb[64 * e:64 * e + 64, :].rearrange(
                    "p (b o) -> p o b", o=OC),
                in1=bc_ps[64 * e:64 * e + 64, 0:64].rearrange(
                    "p (b o) -> p o b", o=OC),
                op=MULT)

        # ---------- P^T = Vblk^T W^T, h-halves pipelined ----------
        pT_ps = ps_big.tile([128, 1024], f32, tag="big", name=f"pT{it}")
        _warm(pT_ps, 14)
        pT_sb = work.tile([128, 1024], f16, tag="pT_sb")
        p_sb = work.tile([128, HC, 128], f16, tag="p_sb")
        for half in range(2):
            hs = slice(512 * half, 512 * half + 512)
            for oc in range(OC):
                nc.tensor.matmul(
                    pT_ps[:, hs],
                    vblk[:, oc, :],
                    wt16[:, oc, hs],
                    start=(oc == 0), stop=(oc == OC - 1),
                    skip_group_check=True)
            _act_copy(nc, pT_sb[:, hs], pT_ps[:, hs])
            nc.sync.dma_start_transpose(p_sb[:, 4 * half:4 * half + 4, :],
                                        pT_sb[:, hs])

        # ---------- upd^T = P^T X (col-tiled per batch) ----------
        u_ps = [ps_u.tile([128, 512], f32, tag="u", name=f"u{it}_{g}")
                for g in range(2)]
        _warm(u_ps[0], 8)
        for g in range(2):
            nc.tensor.matmul(u_ps[g][:], zeros[:], x16[:, 0, 0, 0:512],
                             start=True, stop=False, skip_group_check=True)
        for hcq in range(2):
            for g in range(2):
                for b_ in range(4):
                    b = 4 * g + b_
                    for hc in range(4 * hcq, 4 * hcq + 4):
                        nc.tensor.matmul(
                            u_ps[g][32 * b_:32 * b_ + 16, :],
                            p_sb[:, hc, 16 * b:16 * b + 16],
                            xt16[:, b, hc, :],
                            start=False, stop=(hc == 7),
                            skip_group_check=True,
                            tile_position=(0, 32 * b_))
        # per-group tail: evac, transpose, logits add, softmax -> c16
        u_sb = work.tile([128, 2, 512], f16, tag="u_sb")
        ut = work.tile([128, 2, SC, 128], f16, tag="ut")
        for g in range(2):
            _act_copy(nc, u_sb[:, g, :], u_ps[g][:])
            nc.sync.dma_start_transpose(ut[:, g], u_sb[:, g, :])
            srcu = ut[:, g].rearrange("p sc (b_ jp) -> p sc b_ jp", jp=32)
            nc.vector.tensor_add(
                logits[:, :, 4 * g:4 * g + 4, :],
                logits[:, :, 4 * g:4 * g + 4, :], srcu[:, :, :, 0:16])
            ex = small.tile([128, SC, 4, NCAP], f32, tag="ex")
            nc.scalar.activation(out=ex[:],
                                 in_=logits[:, :, 4 * g:4 * g + 4, :],
                                 func=EXP, scale=1.0, alpha=0.0)
            sm = small.tile([128, SC, 4, 1], f32, tag="sm")
            nc.vector.reduce_sum(sm[:], ex[:], axis=mybir.AxisListType.X)
            rc = small.tile([128, SC, 4, 1], f32, tag="rc")
            nc.vector.reciprocal(rc[:], sm[:])
            nc.vector.tensor_mul(c16[:, :, 4 * g:4 * g + 4, 0:16], ex[:],
                                 rc.broadcast_to([128, SC, 4, NCAP]))
        if dbg is not None and it == 0:
            nc.sync.dma_start(out=dbg["xt"], in_=xt16[:])
            nc.sync.dma_start(out=dbg["usb"], in_=u_sb[:])
            nc.sync.dma_start(out=dbg["ut"], in_=ut[:])
            nc.sync.dma_start(out=dbg["vblk"], in_=vblk[:])
            nc.sync.dma_start(out=dbg["tsb"], in_=t_sb[:])
            nc.sync.dma_start(out=dbg["rs"], in_=rs[:])
            nc.sync.dma_start(out=dbg["psb"], in_=p_sb[:])
            nc.sync.dma_start(out=dbg["logits"], in_=logits[:])
            nc.sync.dma_start(out=dbg["c16"], in_=c16[:])
    ctx.close()


_CACHE = {}


def _host_consts():
    c0 = np.zeros((128, SC, BL, 32), np.float16)
    c0[:, :, :, 0:NCAP] = np.float16(1.0 / NCAP)
    logi = np.zeros((128, SC, BL, NCAP), np.float32)
    vblk0 = np.zeros((128, OC, 128), np.float16)
    ones2 = np.zeros((128, 2), np.float16)
    ones2[0:64, 0] = 1.0
    ones2[64:128, 1] = 1.0
    o2t = np.zeros((2, 128), np.float32)
    o2t[0, 0:64] = 1.0
    o2t[1, 64:128] = 1.0
    zeros = np.zeros((128, 128), np.float16)
    magic = np.full((2, 1), MAGIC, np.int32)
    return {"c0i": c0, "logi": logi, "vblki": vblk0, "ones2": ones2,
            "o2t": o2t, "zeros": zeros, "magic": magic}


def _get_nc(debug_dump=False):
    if "nc" not in _CACHE:
        nc = bacc.Bacc("TRN2", target_bir_lowering=False, debug=False)
        x_d = nc.dram_tensor("x16", [128, BL, SC, 1024], f16,
                             kind="ExternalInput")
        xt_d = nc.dram_tensor("xt16", [128, BL, HC, 512], f16,
                              kind="ExternalInput")
        w_d = nc.dram_tensor("w16", [128, HC, 1024], f16,
                             kind="ExternalInput")
        wt_d = nc.dram_tensor("wt16", [128, OC, 1024], f16,
                              kind="ExternalInput")
        c0_d = nc.dram_tensor("c0i", [128, SC, BL, 32], f16,
                              kind="ExternalInput")
        logits_d = nc.dram_tensor("logi", [128, SC, BL, NCAP], f32,
                                  kind="ExternalInput")
        vblk_d = nc.dram_tensor("vblki", [128, OC, 128], f16,
                                kind="ExternalInput")
        ones2_d = nc.dram_tensor("ones2", [128, 2], f16, kind="ExternalInput")
        o2t_d = nc.dram_tensor("o2t", [2, 128], f32, kind="ExternalInput")
        zeros_d = nc.dram_tensor("zeros", [128, 128], f16,
                                 kind="ExternalInput")
        magic_d = nc.dram_tensor("magic", [2, 1], i32, kind="ExternalInput")
        out_d = nc.dram_tensor("out", [128, 64], f32, kind="ExternalOutput")
        dbg = None
        if debug_dump:
            dbg = {"vblk": nc.dram_tensor("dvblk", [128, OC, 128], f16,
                                          kind="ExternalOutput").ap(),
                   "tsb": nc.dram_tensor("dtsb", [128, 64], f32,
                                         kind="ExternalOutput").ap(),
                   "rs": nc.dram_tensor("drs", [2, 64], f32,
                                        kind="ExternalOutput").ap(),
                   "psb": nc.dram_tensor("dpsb", [128, HC, 128], f16,
                                         kind="ExternalOutput").ap(),
                   "logits": nc.dram_tensor("dlog", [128, SC, BL, NCAP], f32,
                                            kind="ExternalOutput").ap(),
                   "c16": nc.dram_tensor("dc16", [128, SC, BL, 32], f16,
                                         kind="ExternalOutput").ap(),
                   "xt": nc.dram_tensor("dxt", [128, BL, SC, HC, 128], f16,
                                        kind="ExternalOutput").ap(),
                   "usb": nc.dram_tensor("dusb", [128, 2, 512], f16,
                                         kind="ExternalOutput").ap(),
                   "ut": nc.dram_tensor("dut", [128, 2, SC, 128], f16,
                                        kind="ExternalOutput").ap()}
        with tile.TileContext(nc) as tc:
            _build_kernel(tc, out_d.ap(), x_d.ap(), xt_d.ap(), w_d.ap(),
                          wt_d.ap(), c0_d.ap(), logits_d.ap(), vblk_d.ap(),
                          ones2_d.ap(), o2t_d.ap(), zeros_d.ap(),
                          magic_d.ap(), dbg=dbg)
        nc.compile()
        _CACHE["nc"] = nc
    return _CACHE["nc"]


def kernel(inputs: np.ndarray, W: np.ndarray, _trace: bool = False):
    """inputs: (512, 64, 1024) f32; W: (1, 1024, 1024) f32.
    Returns (64, 16, 64) f32."""
    nc = _get_nc()
    consts = _host_consts()
    w0 = W[0].astype(np.float16)
    w16h = np.ascontiguousarray(
        w0.reshape(HC, 128, 1024).transpose(1, 0, 2))
    wt16h = np.ascontiguousarray(
        w0.reshape(1024, OC, 128).transpose(2, 1, 0))
    x16f = inputs.astype(np.float16)              # (S, B, H)
    in_maps = []
    for c in range(N_CORES):
        xs = x16f[:, c * BL:(c + 1) * BL, :]      # (S, BL, H)
        x16h = np.ascontiguousarray(
            xs.reshape(SC, 128, BL, H).transpose(1, 2, 0, 3))
        xt16h = np.ascontiguousarray(
            xs.reshape(S, BL, HC, 128).transpose(3, 1, 2, 0))
        m = {"x16": x16h, "xt16": xt16h, "w16": w16h, "wt16": wt16h}
        m.update(consts)
        in_maps.append(m)
    kw = {}
    if _trace:
        kw = dict(trace=True, trace_cores=[0], stitch_traces=False)
    res = run_bass_kernel_spmd(nc, in_maps, core_ids=list(range(N_CORES)),
                               **kw)
    outs = []
    for c in range(N_CORES):
        v = res.results[c]["out"]          # (128=e*64+d, 64=b*8+oc)
        outs.append(v.reshape(2, 64, BL, 8).transpose(2, 3, 0, 1)
                     .reshape(BL, NCAP, DCAP))
    out = np.concatenate(outs, axis=0)
    if _trace:
        return out.astype(np.float32), res
    return out.astype(np.float32)


# revision 27
# speedup vs baseline: 1.0456x; 1.0456x over previous
"""Trainium2 Bass kernel for capsule dynamic routing (nn_Capsule) — v3.

Reference (per batch item b):
    u = x_b @ W; logits = 0
    for i in 4:
        c = softmax(logits, axis=capsule)
        t_j = sum_s c[s,j] * u[s, j*64:(j+1)*64]; v = squash(t)
        if i < 3: logits[s,j] += u[s, jblk] . v_j

Never materializes u (linearity):
    y_j   = sum_s c[s,j] x_s            y-GEMM   (c stationary, col-tiled)
    t     = W^T y^T                     t-GEMM   (w16 stationary per-slice)
    P^T   = Vblk^T W^T                  P-GEMM   (vblk stationary, block-diag)
    upd^T = P^T X                       upd-GEMM (P slices stationary, col-tiled)

v3 vs v2 (357us) / v1 (335us):
  - squash rsqrt on DVE (bitcast seed + 2 Newton steps, sign-folded into
    the iteration): ScalarE runs only Copy+Exp -> exactly ONE ACT table
    load for the whole kernel (v1/v2 thrashed sqrt|ln<->exp sets).
  - output v transposed on PE before the store so the final DMA writes
    512B-contiguous runs (v1/v2 scattered 4B writes burned ~25us of tail).
  - per-iteration stages split in halves (y/t by h-half, P/upd by h-half,
    u-evac/softmax per batch-group) and emitted interleaved so ScalarE
    evacs + sync-queue DMA transposes overlap PE instead of stalling it.
  - scalar queue issues NO DMA mid-iteration (ACTIVATE only); all
    transposes ride the sync queue; loads use 4 independent staging tiles
    (pool double-buffering raced on HW in v2 - distinct tags only).
  - all input casts f32->f16 on DVE (tensor_copy), not ScalarE.

HW lessons kept:
  - DVE copy PSUM(f32)->SBUF(f16) kills the device; PSUM->f16 casts go
    through ScalarE activation(Copy).
  - matmul start=True lazily zeroes the whole 2KB PSUM bank: accumulation
    groups must own a (partition-range x bank) region exclusively;
    partition-disjoint groups interleave with skip_group_check=True;
    column-disjoint writes into one bank are fine after the first
    start=True (has_written is per-element).
  - PSUM tiles that tiny matmuls write are padded to a full bank so pool
    neighbors never share a bank with an accumulating matmul.
  - nc.vector.memset on f16 tiles is unreliable: constants come from host.
"""
import numpy as np
from contextlib import ExitStack

import concourse.bass as bass
import concourse.bacc as bacc
import concourse.tile as tile
from concourse import mybir
from concourse.bass_utils import run_bass_kernel_spmd

f16 = mybir.dt.float16
f32 = mybir.dt.float32
i32 = mybir.dt.int32
COPY = mybir.ActivationFunctionType.Copy
EXP = mybir.ActivationFunctionType.Exp
MULT = mybir.AluOpType.mult
SUB = mybir.AluOpType.subtract
ADD = mybir.AluOpType.add
SHR = mybir.AluOpType.logical_shift_right

S, B, H = 512, 64, 1024
NCAP, DCAP = 16, 64
ROUTINGS = 4
N_CORES = 8
BL = B // N_CORES          # 8 batch items per core
SC = S // 128              # 4 s-chunks
HC = H // 128              # 8 h-chunks
OC = H // 128              # 8 o-chunks (o = NCAP*DCAP = 1024)
MAGIC = 0x5EF759DF         # rsqrt seed for h = s/2: 0x5f3759df - (1<<22)


def _act_copy(nc, out, in_):
    nc.scalar.activation(out=out, in_=in_, func=COPY, scale=1.0, alpha=0.0)


def _build_kernel(tc, out_d, x_d, xt_d, w_d, wt_d, c0_d, logits_d, vblk_d,
                  ones2_d, o2t_d, zeros_d, magic_d, dbg=None):
    nc = tc.nc
    ctx = ExitStack()
    const = ctx.enter_context(tc.tile_pool(name="const", bufs=1))
    work = ctx.enter_context(tc.tile_pool(name="work", bufs=1))
    small = ctx.enter_context(tc.tile_pool(name="small", bufs=2))
    ps_big = ctx.enter_context(tc.tile_pool(name="ps_big", bufs=2,
                                            space="PSUM"))
    ps_u = ctx.enter_context(tc.tile_pool(name="ps_u", bufs=2, space="PSUM"))
    ps_sm = ctx.enter_context(tc.tile_pool(name="ps_sm", bufs=2,
                                           space="PSUM"))

    # ---------- persistent tensors ----------
    x16 = const.tile([128, BL, SC, 1024], f16)      # (s_loc, b, sc, h)
    xt16 = const.tile([128, BL, HC, 512], f16)    # (h_loc, b, hc, s)
    w16 = const.tile([128, HC, 1024], f16)          # (h_loc, hc, o)
    wt16 = const.tile([128, OC, 1024], f16)         # (o_loc, oc, h)
    c16 = const.tile([128, SC, BL, 32], f16)        # coeffs, cols 16-31 zero
    logits = const.tile([128, SC, BL, NCAP], f32)
    vblk = const.tile([128, OC, 128], f16)          # block-diag v, bj dense
    ones2 = const.tile([128, 2], f16)               # [[1;0],[0;1]] halves
    o2t = const.tile([2, 128], f32)                 # broadcast helper
    zeros = const.tile([128, 128], f16)             # zero-weight psum fill
    magic = const.tile([2, 1], i32)                 # rsqrt seed constant

    # ---------- loads (all layouts host-prepped f16, raw DMAs) ----------
    # everything on the two HWDGE queues (gpsimd DMA completion raced
    # downstream consumers on HW).  zeros first: PE warm-up needs it.
    nc.sync.dma_start(out=zeros[:], in_=zeros_d[:])
    nc.sync.dma_start(out=ones2[:], in_=ones2_d[:])
    nc.sync.dma_start(out=o2t[:], in_=o2t_d[:])
    nc.sync.dma_start(out=magic[:], in_=magic_d[:])
    nc.scalar.dma_start(out=c16[:], in_=c0_d[:])
    nc.scalar.dma_start(out=logits[:], in_=logits_d[:])
    nc.scalar.dma_start(out=vblk[:], in_=vblk_d[:])
    # per-batch chunks so iteration-0 y-GEMM starts as batches land.
    # sync: x b0-3 then w16 then x b4-7? no -- x first on both queues so
    # y finishes early; w/wt right after (t/P of iter 0 need them later).
    for b in range(4):
        nc.sync.dma_start(out=x16[:, b], in_=x_d[:, b])
        nc.scalar.dma_start(out=x16[:, b + 4], in_=x_d[:, b + 4])
    nc.sync.dma_start(out=w16[:], in_=w_d[:])
    nc.scalar.dma_start(out=wt16[:], in_=wt_d[:])
    for b in range(4):
        nc.sync.dma_start(out=xt16[:, b], in_=xt_d[:, b])
        nc.scalar.dma_start(out=xt16[:, b + 4], in_=xt_d[:, b + 4])

    def _warm(ps_tile, n):
        # zero-weight matmuls into a psum tile whose next real producer
        # begins with start=True (which wipes the bank): pure HAM fuel
        # that runs during the dependency-wait gap before the stage.
        for k in range(n):
            nc.tensor.matmul(ps_tile[:, 0:512], zeros[:],
                             x16[:, 0, 0, 0:512],
                             start=(k == 0), stop=False,
                             skip_group_check=True)

    for it in range(ROUTINGS):
        last = it == ROUTINGS - 1

        # ---------- y = C^T X, h-halves pipelined ----------
        y_ps = [ps_big.tile([128, 1024], f32, tag="big", name=f"y{it}_{g}")
                for g in range(2)]
        _warm(y_ps[0], 24 if it == 0 else 10)
        y_sb = work.tile([128, 2, 1024], f16, tag="y_sb")
        yt = work.tile([128, HC, 256], f16, tag="yt")
        yt_dn = work.tile([128, HC, 128], f16, tag="yt_dn")
        ytv = yt.rearrange("p hc (g b_ jp) -> p hc g b_ jp", g=2, jp=32)
        dnv = yt_dn.rearrange("p hc (g b_ j) -> p hc g b_ j", g=2, j=16)
        for half in range(2):
            hs = slice(512 * half, 512 * half + 512)
            for g in range(2):
                for b_ in range(4):
                    b = 4 * g + b_
                    for sc in range(SC):
                        nc.tensor.matmul(
                            y_ps[g][32 * b_:32 * b_ + 32, hs],
                            c16[:, sc, b, :],
                            x16[:, b, sc, hs],
                            start=(sc == 0), stop=(sc == SC - 1),
                            skip_group_check=True,
                            tile_position=(0, 32 * b_))
            for g in range(2):
                _act_copy(nc, y_sb[:, g, hs], y_ps[g][:, hs])
                nc.sync.dma_start_transpose(
                    yt[:, 4 * half:4 * half + 4, 128 * g:128 * g + 128],
                    y_sb[:, g, hs])
            for g in range(2):
                nc.vector.tensor_copy(
                    dnv[:, 4 * half:4 * half + 4, g],
                    ytv[:, 4 * half:4 * half + 4, g, :, 0:16])

        # ---------- t^T = y W: out (bj dense, o), yt stationary ----------
        t_ps = ps_big.tile([128, 1024], f32, tag="big", name=f"t{it}")
        _warm(t_ps, 6)
        t16_sb = work.tile([128, 1024], f16, tag="t16_sb")
        tT = work.tile([128, OC, 128], f16, tag="tT")
        for half in range(2):
            hs = slice(512 * half, 512 * half + 512)
            for hcq in range(2):
                for hc in range(4 * hcq, 4 * hcq + 4):
                    nc.tensor.matmul(
                        t_ps[:, hs],
                        yt_dn[:, hc, :],
                        w16[:, hc, hs],
                        start=(hcq == 0 and hc == 0),
                        stop=(hcq == 1 and hc == 7),
                        skip_group_check=True)
            _act_copy(nc, t16_sb[:, hs], t_ps[:, hs])
            nc.sync.dma_start_transpose(tT[:, 4 * half:4 * half + 4, :],
                                        t16_sb[:, hs])

        # ---------- extract block-diag: t_sb (e*64+d, b*8+oc) ----------
        t_sb = small.tile([128, 64], f16, tag="t_sb")
        dv = t_sb.rearrange("p (b oc) -> p b oc", oc=OC)
        for oc in range(OC):
            tv = tT[:, oc, :].rearrange("p (b j) -> p b j", j=16)
            nc.vector.tensor_copy(dv[0:64, :, oc], tv[0:64, :, 2 * oc])
            nc.vector.tensor_copy(dv[64:128, :, oc],
                                  tv[64:128, :, 2 * oc + 1])

        # ---------- squash: rs = rsqrt(sum_d t^2 + eps) on DVE ----------
        t2 = small.tile([128, 64], f16, tag="t2")
        nc.vector.tensor_mul(t2[:], t_sb[:], t_sb[:])
        sq_ps = ps_sm.tile([2, 512], f32, tag="sm", name=f"sq{it}")
        nc.tensor.matmul(sq_ps[:, 0:64], ones2[:], t2[:])
        h_sb = small.tile([2, 64], f32, tag="h_sb")
        nc.vector.tensor_scalar(out=h_sb[:], in0=sq_ps[:, 0:64],
                                scalar1=0.5, scalar2=5e-8, op0=MULT, op1=ADD)
        ri = small.tile([2, 64], i32, tag="ri")
        nc.vector.tensor_scalar(out=ri[:], in0=h_sb.bitcast(i32),
                                scalar1=1, scalar2=None, op0=SHR)
        r0 = small.tile([2, 64], f32, tag="r0")
        nc.vector.tensor_tensor(out=r0.bitcast(i32),
                                in0=magic.broadcast_to([2, 64]),
                                in1=ri[:], op=SUB)
        # Newton x2 with sign fold: r' = (h*r*r - 1.5) * r  (negates once)
        rr = small.tile([2, 64], f32, tag="rr")
        rs = small.tile([2, 64], f32, tag="rs")
        nc.vector.tensor_mul(rr[:], r0[:], r0[:])
        nc.vector.tensor_mul(rr[:], rr[:], h_sb[:])
        nc.vector.scalar_tensor_tensor(out=rs[:], in0=rr[:], scalar=1.5,
                                       in1=r0[:], op0=SUB, op1=MULT)
        nc.vector.tensor_mul(rr[:], rs[:], rs[:])
        nc.vector.tensor_mul(rr[:], rr[:], h_sb[:])
        nc.vector.scalar_tensor_tensor(out=rs[:], in0=rr[:], scalar=1.5,
                                       in1=rs[:], op0=SUB, op1=MULT)
        # broadcast rs (2,64) -> (128,64): bc[p,n] = rs[p//64, n]
        bc_ps = ps_sm.tile([128, 512], f32, tag="sm", name=f"bc{it}")
        nc.tensor.matmul(bc_ps[:, 0:64], o2t[:], rs[:])

        if last:
            v32 = small.tile([128, 64], f32, tag="v32")
            nc.vector.tensor_mul(v32[:], t_sb[:], bc_ps[:, 0:64])
            # raw dump; host reorders (out[b,2oc+e,d] = v32[e*64+d, b*8+oc])
            nc.sync.dma_start(out=out_d, in_=v32[:])
            break

        # ---------- scatter v = t*bc into block-diag vblk (fused) -------
        # vblk[e*64+d, oc, b*16+2oc+e] = t_sb[e*64+d, b*8+oc] * bc[...]
        pitch = vblk[:].ap[0][0]
        for e in range(2):
            dst = bass.AP(tensor=vblk[:].tensor,
                          offset=64 * e * pitch + e,
                          ap=[[pitch, 64], [130, OC], [16, BL]])
            nc.vector.tensor_tensor(
                out=dst,
                in0=t_sb[64 * e:64 * e + 64, :].rearrange(
                    "p (b o) -> p o b", o=OC),
                in1=bc_ps[64 * e:64 * e + 64, 0:64].rearrange(
                    "p (b o) -> p o b", o=OC),
                op=MULT)

        # ---------- P^T = Vblk^T W^T, h-halves pipelined ----------
        pT_ps = ps_big.tile([128, 1024], f32, tag="big", name=f"pT{it}")
        _warm(pT_ps, 14)
        pT_sb = work.tile([128, 1024], f16, tag="pT_sb")
        p_sb = work.tile([128, HC, 128], f16, tag="p_sb")
        for half in range(2):
            hs = slice(512 * half, 512 * half + 512)
            for oc in range(OC):
                nc.tensor.matmul(
                    pT_ps[:, hs],
                    vblk[:, oc, :],
                    wt16[:, oc, hs],
                    start=(oc == 0), stop=(oc == OC - 1),
                    skip_group_check=True)
            _act_copy(nc, pT_sb[:, hs], pT_ps[:, hs])
            nc.sync.dma_start_transpose(p_sb[:, 4 * half:4 * half + 4, :],
                                        pT_sb[:, hs])

        # ---------- upd^T = P^T X (col-tiled per batch) ----------
        u_ps = [ps_u.tile([128, 512], f32, tag="u", name=f"u{it}_{g}")
                for g in range(2)]
        _warm(u_ps[0], 8)
        for g in range(2):
            nc.tensor.matmul(u_ps[g][:], zeros[:], x16[:, 0, 0, 0:512],
                             start=True, stop=False, skip_group_check=True)
        for hcq in range(2):
            for g in range(2):
                for b_ in range(4):
                    b = 4 * g + b_
                    for hc in range(4 * hcq, 4 * hcq + 4):
                        nc.tensor.matmul(
                            u_ps[g][32 * b_:32 * b_ + 16, :],
                            p_sb[:, hc, 16 * b:16 * b + 16],
                            xt16[:, b, hc, :],
                            start=False, stop=(hc == 7),
                            skip_group_check=True,
                            tile_position=(0, 32 * b_))
        # per-group tail: evac, transpose, logits add, softmax -> c16
        u_sb = work.tile([128, 2, 512], f16, tag="u_sb")
        ut = work.tile([128, 2, SC, 128], f16, tag="ut")
        for g in range(2):
            _act_copy(nc, u_sb[:, g, :], u_ps[g][:])
            nc.sync.dma_start_transpose(ut[:, g], u_sb[:, g, :])
            srcu = ut[:, g].rearrange("p sc (b_ jp) -> p sc b_ jp", jp=32)
            nc.vector.tensor_add(
                logits[:, :, 4 * g:4 * g + 4, :],
                logits[:, :, 4 * g:4 * g + 4, :], srcu[:, :, :, 0:16])
            ex = small.tile([128, SC, 4, NCAP], f32, tag="ex")
            nc.scalar.activation(out=ex[:],
                                 in_=logits[:, :, 4 * g:4 * g + 4, :],
                                 func=EXP, scale=1.0, alpha=0.0)
            sm = small.tile([128, SC, 4, 1], f32, tag="sm")
            nc.vector.reduce_sum(sm[:], ex[:], axis=mybir.AxisListType.X)
            rc = small.tile([128, SC, 4, 1], f32, tag="rc")
            nc.vector.reciprocal(rc[:], sm[:])
            nc.vector.tensor_mul(c16[:, :, 4 * g:4 * g + 4, 0:16], ex[:],
                                 rc.broadcast_to([128, SC, 4, NCAP]))
        if dbg is not None and it == 0:
            nc.sync.dma_start(out=dbg["xt"], in_=xt16[:])
            nc.sync.dma_start(out=dbg["usb"], in_=u_sb[:])
            nc.sync.dma_start(out=dbg["ut"], in_=ut[:])
            nc.sync.dma_start(out=dbg["vblk"], in_=vblk[:])
            nc.sync.dma_start(out=dbg["tsb"], in_=t_sb[:])
            nc.sync.dma_start(out=dbg["rs"], in_=rs[:])
            nc.sync.dma_start(out=dbg["psb"], in_=p_sb[:])
            nc.sync.dma_start(out=dbg["logits"], in_=logits[:])
            nc.sync.dma_start(out=dbg["c16"], in_=c16[:])
    ctx.close()


_CACHE = {}


def _host_consts():
    c0 = np.zeros((128, SC, BL, 32), np.float16)
    c0[:, :, :, 0:NCAP] = np.float16(1.0 / NCAP)
    logi = np.zeros((128, SC, BL, NCAP), np.float32)
    vblk0 = np.zeros((128, OC, 128), np.float16)
    ones2 = np.zeros((128, 2), np.float16)
    ones2[0:64, 0] = 1.0
    ones2[64:128, 1] = 1.0
    o2t = np.zeros((2, 128), np.float32)
    o2t[0, 0:64] = 1.0
    o2t[1, 64:128] = 1.0
    zeros = np.zeros((128, 128), np.float16)
    magic = np.full((2, 1), MAGIC, np.int32)
    return {"c0i": c0, "logi": logi, "vblki": vblk0, "ones2": ones2,
            "o2t": o2t, "zeros": zeros, "magic": magic}


def _get_nc(debug_dump=False):
    if "nc" not in _CACHE:
        nc = bacc.Bacc("TRN2", target_bir_lowering=False, debug=False)
        x_d = nc.dram_tensor("x16", [128, BL, SC, 1024], f16,
                             kind="ExternalInput")
        xt_d = nc.dram_tensor("xt16", [128, BL, HC, 512], f16,
                              kind="ExternalInput")
        w_d = nc.dram_tensor("w16", [128, HC, 1024], f16,
                             kind="ExternalInput")
        wt_d = nc.dram_tensor("wt16", [128, OC, 1024], f16,
                              kind="ExternalInput")
        c0_d = nc.dram_tensor("c0i", [128, SC, BL, 32], f16,
                              kind="ExternalInput")
        logits_d = nc.dram_tensor("logi", [128, SC, BL, NCAP], f32,
                                  kind="ExternalInput")
        vblk_d = nc.dram_tensor("vblki", [128, OC, 128], f16,
                                kind="ExternalInput")
        ones2_d = nc.dram_tensor("ones2", [128, 2], f16, kind="ExternalInput")
        o2t_d = nc.dram_tensor("o2t", [2, 128], f32, kind="ExternalInput")
        zeros_d = nc.dram_tensor("zeros", [128, 128], f16,
                                 kind="ExternalInput")
        magic_d = nc.dram_tensor("magic", [2, 1], i32, kind="ExternalInput")
        out_d = nc.dram_tensor("out", [128, 64], f32, kind="ExternalOutput")
        dbg = None
        if debug_dump:
            dbg = {"vblk": nc.dram_tensor("dvblk", [128, OC, 128], f16,
                                          kind="ExternalOutput").ap(),
                   "tsb": nc.dram_tensor("dtsb", [128, 64], f32,
                                         kind="ExternalOutput").ap(),
                   "rs": nc.dram_tensor("drs", [2, 64], f32,
                                        kind="ExternalOutput").ap(),
                   "psb": nc.dram_tensor("dpsb", [128, HC, 128], f16,
                                         kind="ExternalOutput").ap(),
                   "logits": nc.dram_tensor("dlog", [128, SC, BL, NCAP], f32,
                                            kind="ExternalOutput").ap(),
                   "c16": nc.dram_tensor("dc16", [128, SC, BL, 32], f16,
                                         kind="ExternalOutput").ap(),
                   "xt": nc.dram_tensor("dxt", [128, BL, SC, HC, 128], f16,
                                        kind="ExternalOutput").ap(),
                   "usb": nc.dram_tensor("dusb", [128, 2, 512], f16,
                                         kind="ExternalOutput").ap(),
                   "ut": nc.dram_tensor("dut", [128, 2, SC, 128], f16,
                                        kind="ExternalOutput").ap()}
        with tile.TileContext(nc) as tc:
            _build_kernel(tc, out_d.ap(), x_d.ap(), xt_d.ap(), w_d.ap(),
                          wt_d.ap(), c0_d.ap(), logits_d.ap(), vblk_d.ap(),
                          ones2_d.ap(), o2t_d.ap(), zeros_d.ap(),
                          magic_d.ap(), dbg=dbg)
        nc.compile()
        _CACHE["nc"] = nc
    return _CACHE["nc"]


def kernel(inputs: np.ndarray, W: np.ndarray, _trace: bool = False):
    """inputs: (512, 64, 1024) f32; W: (1, 1024, 1024) f32.
    Returns (64, 16, 64) f32."""
    nc = _get_nc()
    consts = _host_consts()
    w0 = W[0].astype(np.float16)
    w16h = np.ascontiguousarray(
        w0.reshape(HC, 128, 1024).transpose(1, 0, 2))
    wt16h = np.ascontiguousarray(
        w0.reshape(1024, OC, 128).transpose(2, 1, 0))
    x16f = inputs.astype(np.float16)              # (S, B, H)
    in_maps = []
    for c in range(N_CORES):
        xs = x16f[:, c * BL:(c + 1) * BL, :]      # (S, BL, H)
        x16h = np.ascontiguousarray(
            xs.reshape(SC, 128, BL, H).transpose(1, 2, 0, 3))
        xt16h = np.ascontiguousarray(
            xs.reshape(S, BL, HC, 128).transpose(3, 1, 2, 0))
        m = {"x16": x16h, "xt16": xt16h, "w16": w16h, "wt16": wt16h}
        m.update(consts)
        in_maps.append(m)
    kw = {}
    if _trace:
        kw = dict(trace=True, trace_cores=[0], stitch_traces=False)
    res = run_bass_kernel_spmd(nc, in_maps, core_ids=list(range(N_CORES)),
                               **kw)
    outs = []
    for c in range(N_CORES):
        v = res.results[c]["out"]          # (128=e*64+d, 64=b*8+oc)
        outs.append(v.reshape(2, 64, BL, 8).transpose(2, 3, 0, 1)
                     .reshape(BL, NCAP, DCAP))
    out = np.concatenate(outs, axis=0)
    if _trace:
        return out.astype(np.float32), res
    return out.astype(np.float32)
